# revision 22
# baseline (speedup 1.0000x reference)
"""GCN block (2-layer GCNConv + ReLU) on 8 Trainium2 NeuronCores.

This environment charges ~58us of dispatch overhead per STATIC instruction
per run, while For_i hardware-loop iterations execute at native device speed.
The kernel is therefore structured as a handful of For_i loops (~90 static
instructions total):

  - 1D node partitioning: core c owns targets [c*6250, (c+1)*6250), padded to
    49 blocks of 128. Edges (self-loops folded in as ordinary edges) are
    bucketed by target block; every block is padded to the same C chunks of
    128 edge slots so one loop body fits all blocks.
  - Messages are fetched with dma_gather (gpsimd custom gather): the feature
    table is stored as PAIRS of 128-col-padded fp16 rows ([25088, 512B]
    elements) so indices fit int16; a predicated copy selects the right half
    per slot by source-row parity.
  - Aggregation runs feature-major: psum[96, 128] += M_chunk.T @ S_chunk via
    matmul(lhsT=messages[lane, 96], rhs=S[lane, 128]), where S is built on
    DVE from iota==colseg times norm-weight metadata. lhsT cannot take
    register offsets (walrus ldweights), so each chunk is staged into a
    fixed tile first.
  - The 96x96 weight transforms run feature-major over 448-column strips
    (matmul + fused bias/ReLU activation), then a transpose loop (identity
    matmul) re-materializes node-major rows for the next layer's gather
    table, which an AllGather publishes to all cores.
"""

import os
import sys

for _p in ("/opt/trn_rl_repo", "/root/.axon_site/_ro/trn_rl_repo"):
    if os.path.isdir(_p) and _p not in sys.path:
        sys.path.insert(0, _p)

import numpy as np

import concourse.bass as bass
import concourse.bacc as bacc
import concourse.mybir as mybir
import concourse.tile as tile
from concourse import bass_utils
from concourse.bass import ds, ts

F16 = mybir.dt.float16
F32 = mybir.dt.float32
I16 = mybir.dt.int16
U8 = mybir.dt.uint8

P = 128
D = 96
NCORES = 8
N = 50000
NPC = N // NCORES            # 6250 owned nodes per core
NBLK = (NPC + P - 1) // P    # 49 blocks
NPAD = NBLK * P              # 6272 padded nodes per core
NTOT = NCORES * NPAD         # 50176 padded global nodes
NPAIR = NTOT // 2            # 25088 pair rows (fits int16 indices)
TW = 448                     # transform strip width; 14*448 == NPAD


def _preprocess(row, col, ew):
    """Bucket edges (incl. self-loops) by (core, target block); pad every
    block to C chunks of 128 slots; emit per-core gather indices plus
    selection metadata laid out to match dma_gather's slot order (slot j ->
    partition j%128, chunk j//128; index wrap: partition j%16 replicated x8,
    column j//16)."""
    deg = np.bincount(col, weights=ew, minlength=N) + 1.0
    dinv = (1.0 / np.sqrt(deg)).astype(np.float32)
    norm = (dinv[row] * ew * dinv[col]).astype(np.float32)
    selfn = (dinv * dinv).astype(np.float32)

    srcpad = (row // NPC) * NPAD + (row % NPC)  # padded global source ids

    percore = []
    C = 0
    for c in range(NCORES):
        m = (col >= c * NPC) & (col < (c + 1) * NPC)
        t = col[m] - c * NPC
        tself = np.arange(NPC, dtype=np.int64)
        t = np.concatenate([t, tself])
        sg = np.concatenate([srcpad[m], c * NPAD + tself])
        w = np.concatenate([norm[m], selfn[c * NPC + tself]])
        b = t // P
        order = np.argsort(b, kind="stable")
        sg = sg[order]
        tl = (t % P)[order]
        w = w[order]
        cnt = np.bincount(b, minlength=NBLK)
        C = max(C, int(np.ceil(cnt.max() / P)))
        percore.append((sg, tl, w, cnt))

    slots = NBLK * C * P
    idx16 = np.zeros((NCORES, P, NBLK * C * 8), np.int16)
    colseg = np.zeros((NCORES, P, NBLK * C), np.float32)
    wseg = np.zeros((NCORES, P, NBLK * C), np.float32)
    par = np.zeros((NCORES, P, NBLK * C), np.uint8)
    for c in range(NCORES):
        sg, tl, w, cnt = percore[c]
        s_sg = np.zeros(slots, np.int64)
        s_tl = np.zeros(slots, np.int64)
        s_w = np.zeros(slots, np.float32)
        e0 = 0
        for b in range(NBLK):
            n = int(cnt[b])
            o = b * C * P
            s_sg[o:o + n] = sg[e0:e0 + n]
            s_tl[o:o + n] = tl[e0:e0 + n]
            s_w[o:o + n] = w[e0:e0 + n]
            e0 += n
        pair = (s_sg // 2).astype(np.int16)
        parity = (s_sg % 2).astype(np.uint8)
        # slot j of block b -> (partition j%128, chunk j//128)
        colseg[c] = s_tl.reshape(NBLK * C, P).T
        wseg[c] = s_w.reshape(NBLK * C, P).T
        par[c] = parity.reshape(NBLK * C, P).T
        # index wrap: per block, slot j -> (16g + j%16, j//16)
        wrapped = (pair.reshape(NBLK, C * 8, 16).transpose(2, 0, 1)
                   .reshape(16, NBLK * C * 8))
        for g in range(8):
            idx16[c, g * 16:(g + 1) * 16, :] = wrapped
    return C, idx16, colseg, wseg, par


def _build_program(C, repeat=1, no_coll=False):
    nc = bacc.Bacc("TRN2", target_bir_lowering=False, debug=False,
                   enable_asserts=False, num_devices=NCORES)

    xtab_d = nc.dram_tensor("xtab", [NPAIR, 256], F16, kind="ExternalInput").ap()
    idx_d = nc.dram_tensor("idx16", [P, NBLK * C * 8], I16,
                           kind="ExternalInput").ap()
    colseg_d = nc.dram_tensor("colseg", [P, NBLK * C], F32,
                              kind="ExternalInput").ap()
    wseg_d = nc.dram_tensor("wseg", [P, NBLK * C], F32,
                            kind="ExternalInput").ap()
    par_d = nc.dram_tensor("par", [P, NBLK * C], U8, kind="ExternalInput").ap()
    iotaf_d = nc.dram_tensor("iotaf", [P, P], F32, kind="ExternalInput").ap()
    ident_d = nc.dram_tensor("ident", [P, P], F16, kind="ExternalInput").ap()
    w1_d = nc.dram_tensor("w1", [D, D], F16, kind="ExternalInput").ap()
    w2_d = nc.dram_tensor("w2", [D, D], F16, kind="ExternalInput").ap()
    b1_d = nc.dram_tensor("b1", [D, 1], F32, kind="ExternalInput").ap()
    b2_d = nc.dram_tensor("b2", [D, 1], F32, kind="ExternalInput").ap()
    out_d = nc.dram_tensor("out", [NPAD, D], F32, kind="ExternalOutput").ap()

    with tile.TileContext(nc) as tc:
        with (
            tc.tile_pool(name="meta", bufs=1) as mp,
            tc.tile_pool(name="work", bufs=1) as wp,
            tc.tile_pool(name="pagg", bufs=1, space="PSUM") as pagg,
            tc.tile_pool(name="ptr", bufs=1, space="PSUM") as ptrp,
            tc.tile_pool(name="pz", bufs=2, space="PSUM") as pzp,
            tc.tile_pool(name="dram", bufs=1, space="DRAM") as dp,
        ):
            idx_sb = mp.tile([P, NBLK * C * 8], I16, tag="idx")
            colseg_sb = mp.tile([P, NBLK * C], F32, tag="colseg")
            wseg_sb = mp.tile([P, NBLK * C], F32, tag="wseg")
            par_sb = mp.tile([P, NBLK * C], U8, tag="par")
            iotaf_sb = mp.tile([P, P], F32, tag="iotaf")
            ident_sb = mp.tile([P, P], F16, tag="ident")
            w1_sb = mp.tile([D, D], F16, tag="w1")
            w2_sb = mp.tile([D, D], F16, tag="w2")
            b1_sb = mp.tile([D, 1], F32, tag="b1")
            b2_sb = mp.tile([D, 1], F32, tag="b2")
            for sb, d in ((idx_sb, idx_d), (colseg_sb, colseg_d),
                          (wseg_sb, wseg_d), (par_sb, par_d),
                          (iotaf_sb, iotaf_d), (ident_sb, ident_d),
                          (w1_sb, w1_d), (w2_sb, w2_d), (b1_sb, b1_d),
                          (b2_sb, b2_d)):
                nc.sync.dma_start(sb[:], d[:])

            ia = iotaf_sb[:]

            # [128, 128] staging tile for the pre-transpose feature block;
            # rows 96..127 are zeroed once and only [:96] is ever rewritten,
            # so the transpose matmuls see zero padding.
            fstage = wp.tile([P, P], F16, tag="fstage")
            nc.vector.tensor_scalar(
                out=fstage[:], in0=ident_sb[:], scalar1=-7.0,
                scalar2=None, op0=mybir.AluOpType.is_equal)

            aggs = wp.tile([D, P], F16, tag="aggs")
            gb = wp.tile([P, C * 256], F16, tag="gb")
            msg = wp.tile([P, C * D], F16, tag="msg")
            smat = wp.tile([P, C * P], F16, tag="smat")
            h1s = wp.tile([D, P], F16, tag="h1s")
            t2n = wp.tile([P, D], F32, tag="t2n")
            t2n16 = wp.tile([P, D], F16, tag="t2n16")
            psum = pagg.tile([D, P], F32, tag="psum")
            ptr = ptrp.tile([P, P], F32, tag="ptr")

            t2own = dp.tile([NPAD, P], F16, tag="t2own", name="t2own")
            t2fulls = [
                dp.tile([NTOT, P], F16, tag=f"t2full{r}", addr_space="Shared",
                        name=f"t2full{r}")
                for r in range(repeat)
            ]

            def agg_layer(table_pairs_ap, post_block):
                with tc.For_i(0, NBLK) as b:
                    nc.gpsimd.dma_gather(
                        out_ap=gb[:].rearrange("p (c e) -> p c e", e=256),
                        in_ap=table_pairs_ap,
                        idxs_ap=idx_sb[:, ts(b, C * 8)],
                        num_idxs=C * P,
                        num_idxs_reg=C * P,
                        elem_size=256,
                        single_packet=False,
                    )
                    g3 = gb[:].rearrange("p (c e) -> p c e", e=256)
                    m3 = msg[:].rearrange("p (c f) -> p c f", f=D)
                    pa = par_sb[:, ts(b, C)]
                    pmask = bass.AP(pa.tensor, pa.offset,
                                    [list(pa.ap[0]), list(pa.ap[1]), [0, D]])
                    nc.vector.tensor_copy(m3, g3[:, :, 0:D])
                    nc.vector.copy_predicated(m3, pmask, g3[:, :, 128:128 + D])
                    s3 = smat[:].rearrange("p (c m) -> p c m", m=P)
                    cs = colseg_sb[:, ts(b, C)]
                    ap_cs = bass.AP(cs.tensor, cs.offset,
                                    [list(cs.ap[0]), list(cs.ap[1]), [0, P]])
                    ws = wseg_sb[:, ts(b, C)]
                    ap_ws = bass.AP(ws.tensor, ws.offset,
                                    [list(ws.ap[0]), list(ws.ap[1]), [0, P]])
                    ap_i2 = bass.AP(ia.tensor, ia.offset,
                                    [list(ia.ap[0]), [0, C], [1, P]])
                    nc.vector.tensor_tensor(out=s3, in0=ap_i2, in1=ap_cs,
                                            op=mybir.AluOpType.is_equal)
                    nc.vector.tensor_tensor(out=s3, in0=s3, in1=ap_ws,
                                            op=mybir.AluOpType.mult)
                    for cc in range(C):
                        nc.tensor.matmul(out=psum[:],
                                         lhsT=msg[:, cc * D:(cc + 1) * D],
                                         rhs=smat[:, cc * P:(cc + 1) * P],
                                         start=(cc == 0), stop=(cc == C - 1),
                                         skip_group_check=True)
                    nc.vector.tensor_copy(aggs[:], psum[:])
                    post_block(b)

            def post1(b):
                # h1 = relu(agg@W1+b1); t2 = h1@W2; transpose to node-major
                # rows; publish the own slice of the layer-2 table
                pz1 = pzp.tile([D, P], F32, tag="pz1", name="pz1")
                nc.tensor.matmul(out=pz1[:], lhsT=w1_sb[:], rhs=aggs[:],
                                 start=True, stop=True)
                nc.scalar.activation(h1s[:], pz1[:],
                                     mybir.ActivationFunctionType.Relu,
                                     bias=b1_sb[:], scale=1.0)
                pz2 = pzp.tile([D, P], F32, tag="pz2", name="pz2")
                nc.tensor.matmul(out=pz2[:], lhsT=w2_sb[:],
                                 rhs=h1s[:], start=True, stop=True)
                nc.vector.tensor_copy(fstage[:D, :], pz2[:])
                nc.tensor.matmul(out=ptr[:], lhsT=fstage[:],
                                 rhs=ident_sb[:], start=True, stop=True)
                nc.vector.tensor_copy(t2n16[:], ptr[:, :D])
                nc.sync.dma_start(t2own[ds(b * P, P), :D], t2n16[:])

            def post2(b):
                # relu(agg2 + b2), transpose, write out
                nc.scalar.activation(fstage[:D, :], aggs[:],
                                     mybir.ActivationFunctionType.Relu,
                                     bias=b2_sb[:], scale=1.0)
                nc.tensor.matmul(out=ptr[:], lhsT=fstage[:],
                                 rhs=ident_sb[:], start=True, stop=True)
                nc.vector.tensor_copy(t2n[:], ptr[:, :D])
                nc.sync.dma_start(out_d[ds(b * P, P), :], t2n[:])

            for rep in range(repeat):
                t2full = t2fulls[rep]
                agg_layer(xtab_d, post1)
                if not no_coll:
                    nc.gpsimd.collective_compute(
                        "AllGather", mybir.AluOpType.bypass,
                        replica_groups=[list(range(NCORES))],
                        ins=[t2own[:]], outs=[t2full[:]])
                t2pairs = t2full[:].rearrange("(r h) e -> r (h e)", h=2)
                agg_layer(t2pairs, post2)

    nc.compile()
    return nc


_CACHE = {}


def _get_program(C, repeat=1, no_coll=False):
    key = (C, repeat, no_coll)
    if key not in _CACHE:
        _CACHE[key] = _build_program(C, repeat=repeat, no_coll=no_coll)
    return _CACHE[key]


def _make_inputs(x, W1, b1, W2, b2, pre):
    C, idx16, colseg, wseg, par = pre
    xn = np.zeros((NTOT, P), np.float16)
    xr = np.asarray(x, np.float32).astype(np.float16)
    for c in range(NCORES):
        xn[c * NPAD: c * NPAD + NPC, :D] = xr[c * NPC:(c + 1) * NPC]
    xtab = xn.reshape(NPAIR, 256)
    common = {
        "xtab": xtab,
        "iotaf": np.tile(np.arange(P, dtype=np.float32), (P, 1)),
        "ident": np.eye(P, dtype=np.float16),
        "w1": np.asarray(W1, np.float32).astype(np.float16),
        "w2": np.asarray(W2, np.float32).astype(np.float16),
        "b1": np.asarray(b1, np.float32).reshape(D, 1),
        "b2": np.asarray(b2, np.float32).reshape(D, 1),
    }
    in_maps = []
    for c in range(NCORES):
        m = dict(common)
        m["idx16"] = idx16[c]
        m["colseg"] = colseg[c]
        m["wseg"] = wseg[c]
        m["par"] = par[c]
        in_maps.append(m)
    return in_maps


def kernel(x, edge_index, edge_weight, batch, W1, b1, W2, b2, **_unused):
    x = np.asarray(x, dtype=np.float32)
    edge_index = np.asarray(edge_index)
    ew = np.asarray(edge_weight, dtype=np.float32)
    row = np.asarray(edge_index[0], dtype=np.int64)
    col = np.asarray(edge_index[1], dtype=np.int64)

    pre = _preprocess(row, col, ew)
    nc = _get_program(pre[0])
    in_maps = _make_inputs(x, W1, b1, W2, b2, pre)

    res = bass_utils.run_bass_kernel_spmd(nc, in_maps,
                                          core_ids=list(range(NCORES)))
    out = np.concatenate([res.results[c]["out"][:NPC] for c in range(NCORES)],
                         axis=0)
    return out.astype(np.float32)


# revision 30
# speedup vs baseline: 2.3658x; 2.3658x over previous
"""GCN block (2-layer GCNConv + ReLU) on 8 Trainium2 NeuronCores.

This environment charges ~58us of dispatch overhead per STATIC instruction
per run, while For_i hardware-loop iterations execute at native device speed.
The kernel is therefore structured as a handful of For_i loops (~90 static
instructions total):

  - 1D node partitioning: core c owns targets [c*6250, (c+1)*6250), padded to
    49 blocks of 128. Edges (self-loops folded in as ordinary edges) are
    bucketed by target block; every block is padded to the same C chunks of
    128 edge slots so one loop body fits all blocks.
  - Messages are fetched with dma_gather (gpsimd custom gather): the feature
    table is stored as PAIRS of 128-col-padded fp16 rows ([25088, 512B]
    elements) so indices fit int16; a predicated copy selects the right half
    per slot by source-row parity.
  - Aggregation runs feature-major: psum[96, 128] += M_chunk.T @ S_chunk via
    matmul(lhsT=messages[lane, 96], rhs=S[lane, 128]), where S is built on
    DVE from iota==colseg times norm-weight metadata. lhsT cannot take
    register offsets (walrus ldweights), so each chunk is staged into a
    fixed tile first.
  - The 96x96 weight transforms run feature-major over 448-column strips
    (matmul + fused bias/ReLU activation), then a transpose loop (identity
    matmul) re-materializes node-major rows for the next layer's gather
    table, which an AllGather publishes to all cores.
"""

import os
import sys

for _p in ("/opt/trn_rl_repo", "/root/.axon_site/_ro/trn_rl_repo"):
    if os.path.isdir(_p) and _p not in sys.path:
        sys.path.insert(0, _p)

import numpy as np

import concourse.bass as bass
import concourse.bacc as bacc
import concourse.mybir as mybir
import concourse.tile as tile
from concourse import bass_utils
from concourse.bass import ds, ts

F16 = mybir.dt.float16
F32 = mybir.dt.float32
I16 = mybir.dt.int16
U8 = mybir.dt.uint8

P = 128
D = 96
NCORES = 8
N = 50000
NPC = N // NCORES            # 6250 owned nodes per core
NBLK = (NPC + P - 1) // P    # 49 blocks
NPAD = NBLK * P              # 6272 padded nodes per core
NTOT = NCORES * NPAD         # 50176 padded global nodes
NPAIR = NTOT // 2            # 25088 pair rows (fits int16 indices)
TW = 448                     # transform strip width; 14*448 == NPAD


def _preprocess(row, col, ew):
    """Bucket edges (incl. self-loops) by (core, target block); pad every
    block to C chunks of 128 slots; emit per-core gather indices plus
    selection metadata laid out to match dma_gather's slot order (slot j ->
    partition j%128, chunk j//128; index wrap: partition j%16 replicated x8,
    column j//16)."""
    deg = np.bincount(col, weights=ew, minlength=N) + 1.0
    dinv = (1.0 / np.sqrt(deg)).astype(np.float32)
    norm = (dinv[row] * ew * dinv[col]).astype(np.float32)
    selfn = (dinv * dinv).astype(np.float32)

    # Degree-balanced block assignment (LPT): per core, pack targets into 49
    # blocks of <=128 so per-block edge counts (incl self-loop) equalize;
    # minimizes C = max chunks per block. slot[t] = block*128 + lane.
    ecnt = np.bincount(col, minlength=N) + 1  # edges + self per target
    t2slot = np.zeros(N, np.int64)
    for c in range(NCORES):
        cn = ecnt[c * NPC:(c + 1) * NPC]
        order = np.argsort(-cn, kind="stable")
        load = np.zeros(NBLK, np.int64)
        fill = np.zeros(NBLK, np.int64)
        slot = np.zeros(NPC, np.int64)
        for t in order:
            avail = np.flatnonzero(fill < P)
            b = avail[np.argmin(load[avail])]
            slot[t] = b * P + fill[b]
            fill[b] += 1
            load[b] += cn[t]
        t2slot[c * NPC:(c + 1) * NPC] = slot

    srcpad = (row // NPC) * NPAD + t2slot[row]  # padded global source slots

    percore = []
    C = 0
    for c in range(NCORES):
        m = (col >= c * NPC) & (col < (c + 1) * NPC)
        t = t2slot[col[m]]                      # local slot of target
        tself = np.arange(NPC, dtype=np.int64)
        t = np.concatenate([t, t2slot[c * NPC + tself]])
        sg = np.concatenate([srcpad[m], c * NPAD + t2slot[c * NPC + tself]])
        w = np.concatenate([norm[m], selfn[c * NPC + tself]])
        b = t // P
        order = np.argsort(b, kind="stable")
        sg = sg[order]
        tl = (t % P)[order]
        w = w[order]
        cnt = np.bincount(b, minlength=NBLK)
        C = max(C, int(np.ceil(cnt.max() / P)))
        percore.append((sg, tl, w, cnt))

    slots = NBLK * C * P
    idx16 = np.zeros((NCORES, P, NBLK * C * 8), np.int16)
    colseg = np.zeros((NCORES, P, NBLK * C), np.float32)
    wseg = np.zeros((NCORES, P, NBLK * C), np.float32)
    par = np.zeros((NCORES, P, NBLK * C), np.uint8)
    for c in range(NCORES):
        sg, tl, w, cnt = percore[c]
        s_sg = np.zeros(slots, np.int64)
        s_tl = np.zeros(slots, np.int64)
        s_w = np.zeros(slots, np.float32)
        e0 = 0
        for b in range(NBLK):
            n = int(cnt[b])
            o = b * C * P
            s_sg[o:o + n] = sg[e0:e0 + n]
            s_tl[o:o + n] = tl[e0:e0 + n]
            s_w[o:o + n] = w[e0:e0 + n]
            e0 += n
        pair = (s_sg // 2).astype(np.int16)
        parity = (s_sg % 2).astype(np.uint8)
        # slot j of block b -> (partition j%128, chunk j//128)
        colseg[c] = s_tl.reshape(NBLK * C, P).T
        wseg[c] = s_w.reshape(NBLK * C, P).T
        par[c] = parity.reshape(NBLK * C, P).T
        # index wrap: per block, slot j -> (16g + j%16, j//16)
        wrapped = (pair.reshape(NBLK, C * 8, 16).transpose(2, 0, 1)
                   .reshape(16, NBLK * C * 8))
        for g in range(8):
            idx16[c, g * 16:(g + 1) * 16, :] = wrapped
    return C, idx16, colseg, wseg, par, t2slot


def _pack_layout(C):
    """Byte layout of the packed per-core metadata/constants tensor."""
    sections = [
        ("colseg", 4 * NBLK * C),
        ("wseg", 4 * NBLK * C),
        ("b1", 4),
        ("b2", 4),
        ("iotaf", 4 * P),
        ("idx16", 2 * NBLK * C * 8),
        ("ident", 2 * P),
        ("w1", 2 * D),
        ("w2", 2 * D),
        ("par", NBLK * C),
    ]
    offs = {}
    o = 0
    for name, nb in sections:
        offs[name] = (o, nb)
        o += nb
    tot = (o + 3) // 4 * 4
    return offs, tot


def _build_program(C, repeat=1, no_coll=False, trip=NBLK, parts=('gather','select','build','mm','post')):
    nc = bacc.Bacc("TRN2", target_bir_lowering=False, debug=False,
                   enable_asserts=False, num_devices=NCORES)

    xtab_d = nc.dram_tensor("xtab", [NPAIR, 256], F16, kind="ExternalInput").ap()
    offs, tot = _pack_layout(C)
    pack_d = nc.dram_tensor("pack", [P, tot], U8, kind="ExternalInput").ap()
    out_d = nc.dram_tensor("out", [NPAD, D], F32, kind="ExternalOutput").ap()

    with tile.TileContext(nc) as tc:
        with (
            tc.tile_pool(name="meta", bufs=1) as mp,
            tc.tile_pool(name="work", bufs=1) as wp,
            tc.tile_pool(name="pagg", bufs=1, space="PSUM") as pagg,
            tc.tile_pool(name="ptr", bufs=1, space="PSUM") as ptrp,
            tc.tile_pool(name="pz", bufs=2, space="PSUM") as pzp,
            tc.tile_pool(name="dram", bufs=1, space="DRAM") as dp,
        ):
            pack_sb = mp.tile([P, tot], U8, tag="pack")
            nc.sync.dma_start(pack_sb[:], pack_d[:])

            def view(name, dt, n, rows=P):
                o, nb = offs[name]
                return pack_sb[:rows, o:o + nb].bitcast(dt)

            colseg_sb = view("colseg", F32, NBLK * C)
            wseg_sb = view("wseg", F32, NBLK * C)
            b1_sb = view("b1", F32, 1, rows=D)
            b2_sb = view("b2", F32, 1, rows=D)
            iotaf_sb = view("iotaf", F32, P)
            idx_sb = view("idx16", I16, NBLK * C * 8)
            ident_sb = view("ident", F16, P)
            w1_sb = view("w1", F16, D, rows=D)
            w2_sb = view("w2", F16, D, rows=D)
            par_sb = view("par", U8, NBLK * C)

            ia = iotaf_sb

            # [128, 128] staging tile for the pre-transpose feature block;
            # rows 96..127 are zeroed once and only [:96] is ever rewritten,
            # so the transpose matmuls see zero padding.
            fstage = wp.tile([P, P], F16, tag="fstage")
            nc.vector.tensor_scalar(
                out=fstage[:], in0=ident_sb, scalar1=-7.0,
                scalar2=None, op0=mybir.AluOpType.is_equal)

            aggs = wp.tile([D, P], F16, tag="aggs")
            gb = wp.tile([P, C * 256], F16, tag="gb")
            msg = wp.tile([P, C * D], F16, tag="msg")
            smat = wp.tile([P, C * P], F16, tag="smat")
            h1s = wp.tile([D, P], F16, tag="h1s")
            t2n = wp.tile([P, D], F32, tag="t2n")
            t2n16 = wp.tile([P, D], F16, tag="t2n16")
            psum = pagg.tile([D, P], F32, tag="psum")
            ptr = ptrp.tile([P, P], F32, tag="ptr")

            t2own = dp.tile([NPAD, P], F16, tag="t2own", name="t2own")
            t2fulls = [
                dp.tile([NTOT, P], F16, tag=f"t2full{r}", addr_space="Shared",
                        name=f"t2full{r}")
                for r in range(repeat)
            ]

            def agg_layer(table_pairs_ap, post_block):
                with tc.For_i(0, trip) as b:
                    if 'gather' in parts:
                        nc.gpsimd.dma_gather(
                            out_ap=gb[:].rearrange("p (c e) -> p c e", e=256),
                            in_ap=table_pairs_ap,
                            idxs_ap=idx_sb[:, ts(b, C * 8)],
                            num_idxs=C * P,
                            num_idxs_reg=C * P,
                            elem_size=256,
                            single_packet=False,
                        )
                    g3 = gb[:].rearrange("p (c e) -> p c e", e=256)
                    m3 = msg[:].rearrange("p (c f) -> p c f", f=D)
                    pa = par_sb[:, ts(b, C)]
                    pmask = bass.AP(pa.tensor, pa.offset,
                                    [list(pa.ap[0]), list(pa.ap[1]), [0, D]])
                    if 'select' in parts:
                        nc.vector.tensor_copy(m3, g3[:, :, 0:D])
                        nc.vector.copy_predicated(m3, pmask,
                                                  g3[:, :, 128:128 + D])
                    s3 = smat[:].rearrange("p (c m) -> p c m", m=P)
                    cs = colseg_sb[:, ts(b, C)]
                    ap_cs = bass.AP(cs.tensor, cs.offset,
                                    [list(cs.ap[0]), list(cs.ap[1]), [0, P]])
                    ws = wseg_sb[:, ts(b, C)]
                    ap_ws = bass.AP(ws.tensor, ws.offset,
                                    [list(ws.ap[0]), list(ws.ap[1]), [0, P]])
                    ap_i2 = bass.AP(ia.tensor, ia.offset,
                                    [list(ia.ap[0]), [0, C], [1, P]])
                    if 'build' in parts:
                        nc.vector.tensor_tensor(out=s3, in0=ap_i2, in1=ap_cs,
                                                op=mybir.AluOpType.is_equal)
                        nc.vector.tensor_tensor(out=s3, in0=s3, in1=ap_ws,
                                                op=mybir.AluOpType.mult)
                    if 'mm' in parts:
                        for cc in range(C):
                            nc.tensor.matmul(
                                out=psum[:],
                                lhsT=msg[:, cc * D:(cc + 1) * D],
                                rhs=smat[:, cc * P:(cc + 1) * P],
                                start=(cc == 0), stop=(cc == C - 1),
                                skip_group_check=True)
                        nc.vector.tensor_copy(aggs[:], psum[:])
                    if 'post' in parts:
                        post_block(b)
                    elif 'dma' in parts:
                        nc.sync.dma_start(out_d[ds(b * P, P), :], t2n[:])

            def post1(b):
                # h1 = relu(agg@W1+b1) feature-major, then
                # t2_node[t, m] = sum_n h1[n, t] W2[n, m] -- the transform
                # and the node-major transpose in one matmul.
                pz1 = pzp.tile([D, P], F32, tag="pz1", name="pz1")
                nc.tensor.matmul(out=pz1[:], lhsT=w1_sb, rhs=aggs[:],
                                 start=True, stop=True)
                nc.scalar.activation(h1s[:], pz1[:],
                                     mybir.ActivationFunctionType.Relu,
                                     bias=b1_sb, scale=1.0)
                pz2 = pzp.tile([P, D], F32, tag="pz2", name="pz2")
                nc.tensor.matmul(out=pz2[:], lhsT=h1s[:],
                                 rhs=w2_sb, start=True, stop=True)
                nc.vector.tensor_copy(t2n16[:], pz2[:])
                nc.sync.dma_start(t2own[ds(b * P, P), :D], t2n16[:])

            def post2(b):
                # relu(agg2 + b2), transpose, write out
                nc.scalar.activation(fstage[:D, :], aggs[:],
                                     mybir.ActivationFunctionType.Relu,
                                     bias=b2_sb, scale=1.0)
                nc.tensor.matmul(out=ptr[:], lhsT=fstage[:],
                                 rhs=ident_sb, start=True, stop=True)
                nc.vector.tensor_copy(t2n[:], ptr[:, :D])
                nc.sync.dma_start(out_d[ds(b * P, P), :], t2n[:])

            for rep in range(repeat):
                t2full = t2fulls[rep]
                agg_layer(xtab_d, post1)
                if not no_coll:
                    nc.gpsimd.collective_compute(
                        "AllGather", mybir.AluOpType.bypass,
                        replica_groups=[list(range(NCORES))],
                        ins=[t2own[:]], outs=[t2full[:]])
                if 'dummy2' in parts:
                    agg_layer(xtab_d, post2)
                else:
                    t2pairs = t2full[:].rearrange("(r h) e -> r (h e)", h=2)
                    agg_layer(t2pairs, post2)

    nc.compile()
    return nc


_CACHE = {}


def _get_program(C, repeat=1, no_coll=False, trip=NBLK,
                 parts=('gather', 'select', 'build', 'mm', 'post')):
    key = (C, repeat, no_coll, trip, tuple(parts))
    if key not in _CACHE:
        _CACHE[key] = _build_program(C, repeat=repeat, no_coll=no_coll,
                                     trip=trip, parts=parts)
    return _CACHE[key]


def _make_inputs(x, W1, b1, W2, b2, pre):
    C, idx16, colseg, wseg, par, t2slot = pre
    xn = np.zeros((NTOT, P), np.float16)
    xr = np.asarray(x, np.float32).astype(np.float16)
    for c in range(NCORES):
        xn[c * NPAD + t2slot[c * NPC:(c + 1) * NPC], :D] = \
            xr[c * NPC:(c + 1) * NPC]
    xtab = xn.reshape(NPAIR, 256)
    offs, tot = _pack_layout(C)

    def pad128(a):
        out = np.zeros((P,) + a.shape[1:], a.dtype)
        out[:a.shape[0]] = a
        return out

    parts128 = {
        "colseg": None, "wseg": None, "idx16": None, "par": None,
        "b1": pad128(np.asarray(b1, np.float32).reshape(D, 1)),
        "b2": pad128(np.asarray(b2, np.float32).reshape(D, 1)),
        "iotaf": np.tile(np.arange(P, dtype=np.float32), (P, 1)),
        "ident": np.eye(P, dtype=np.float16),
        "w1": pad128(np.asarray(W1, np.float32).astype(np.float16)),
        "w2": pad128(np.asarray(W2, np.float32).astype(np.float16)),
    }
    in_maps = []
    for c in range(NCORES):
        parts128["colseg"] = colseg[c]
        parts128["wseg"] = wseg[c]
        parts128["idx16"] = idx16[c]
        parts128["par"] = par[c]
        pack = np.zeros((P, tot), np.uint8)
        for name, (o, nb) in offs.items():
            a = np.ascontiguousarray(parts128[name]).view(np.uint8)
            a = a.reshape(P, -1)
            assert a.shape[1] == nb, (name, a.shape, nb)
            pack[:, o:o + nb] = a
        in_maps.append({"xtab": xtab, "pack": pack.copy()})
    return in_maps


def kernel(x, edge_index, edge_weight, batch, W1, b1, W2, b2, **_unused):
    x = np.asarray(x, dtype=np.float32)
    edge_index = np.asarray(edge_index)
    ew = np.asarray(edge_weight, dtype=np.float32)
    row = np.asarray(edge_index[0], dtype=np.int64)
    col = np.asarray(edge_index[1], dtype=np.int64)

    pre = _preprocess(row, col, ew)
    nc = _get_program(pre[0])
    in_maps = _make_inputs(x, W1, b1, W2, b2, pre)

    res = bass_utils.run_bass_kernel_spmd(nc, in_maps,
                                          core_ids=list(range(NCORES)))
    t2slot = pre[5]
    out = np.concatenate(
        [res.results[c]["out"][t2slot[c * NPC:(c + 1) * NPC]]
         for c in range(NCORES)], axis=0)
    return out.astype(np.float32)
